# revision 5
# baseline (speedup 1.0000x reference)
"""Trainium2 Bass kernel: Conv2d(1->64, k=7, valid) on data [32,1,224,224] f32.

Data-parallel over batch (4 images per core on 8 cores).  Per core:
im2col matmul in fp16 (fp32 PSUM, K=49), built to minimize DMA-engine
time (16 engines/core; big 25KB packets stream ~25GB/s/engine while
12.5KB packets degrade under load) and keep the PE's two 64-col row
groups streaming concurrently (sustained PE rate is duty-cycle
throttled to ~half, so avoid extra column streaming entirely).

Layout/pipeline (per core, 16 row-block "tiles" of 56 output rows,
processed as 8 pairs):
  - host: builds the full im2col in DRAM: per tile 49 rows (7 ky x
    7 kx shifts), each a contiguous run of 56*224+8 fp16 elements
    (25KB packets, clean sequential reads).
  - i2c: one contiguous [49, RUN] DMA per tile from DRAM.  Pair
    layout: tile A at partitions 0..48 (PE row group h0), tile B at
    64..112 (h1).
  - matmul: per 448-col PSUM chunk, 2 concurrent matmuls: A -> ps[0:64]
    and B -> ps[64:128] (different row groups dual-issue on the PE).
  - copy: psum [128,448] f32 -> ob fp16, alternating DVE/ACT.
  - out: fp16 stores (no cast) on the sync HWDGE queue, one DMA per
    tile [64ch, nrows*224] (25KB descriptors).  Cols 218..223 are
    garbage (kx wrap) and are sliced off on the host, which also does
    the lossless fp16->fp32 cast of the result.
"""

import numpy as np

B = 32            # full batch
OC = 64           # out channels
KS = 7            # kernel size
H = 224           # input H=W
OH = 218          # valid output rows/cols
OW = 224          # computed output width (incl 6 garbage cols)
NCORES = 8
IPC = B // NCORES  # images per core

BLK = 56          # output rows per tile
NBLK = 4          # tiles per image (3x56 + 1x50 valid rows)
NTILES = IPC * NBLK
NPAIRS = NTILES // 2
NCOLS = BLK * OW  # 12544 im2col columns per tile
RUN = NCOLS + 8   # per-partition run (covers kx shifts)
CHUNK = 448       # psum chunk columns
NCHUNK = NCOLS // CHUNK  # 28

KP = KS * KS      # 49 im2col partitions per tile

_CACHE = {}


def _build():
    import concourse.mybir as mybir
    import concourse.tile as tile
    from concourse import bacc

    nc = bacc.Bacc("TRN2", target_bir_lowering=False, debug=False)

    i2cd = nc.dram_tensor("i2cd", [NTILES, KP, RUN], mybir.dt.float16,
                          kind="ExternalInput")
    wbd = nc.dram_tensor("wbd", [2, KP, OC], mybir.dt.float16,
                         kind="ExternalInput")
    out = nc.dram_tensor("out", [IPC, OC, OH, OW], mybir.dt.float16,
                         kind="ExternalOutput")

    with tile.TileContext(nc) as tc:
        with (
            tc.tile_pool(name="wp", bufs=1) as w_pool,
            tc.tile_pool(name="i2c", bufs=3) as i2c_pool,
            tc.tile_pool(name="ob", bufs=3) as ob_pool,
            tc.tile_pool(name="ps", bufs=8, space="PSUM") as ps_pool,
        ):
            # weights: row group h0 (rows 0..48) for tile A, h1 (64..112)
            # for tile B.
            wt = w_pool.tile([128, OC], mybir.dt.float16)
            nc.scalar.dma_start(out=wt[0:KP, :], in_=wbd[0, :, :])
            nc.scalar.dma_start(out=wt[64:64 + KP, :], in_=wbd[1, :, :])

            for q in range(NPAIRS):
                i2c = i2c_pool.tile([128, RUN], mybir.dt.float16,
                                    tag="i2c", name=f"i2c{q}")
                for half in range(2):
                    nc.gpsimd.dma_start(
                        out=i2c[64 * half:64 * half + KP, :],
                        in_=i2cd[2 * q + half, :, :])

                ob = ob_pool.tile([128, NCOLS], mybir.dt.float16, tag="ob")
                for j in range(NCHUNK):
                    ps = ps_pool.tile([128, CHUNK], mybir.dt.float32,
                                      tag="ps")
                    c0 = CHUNK * j
                    nc.tensor.matmul(
                        ps[0:OC, :], wt[0:KP, :],
                        i2c[0:KP, c0:c0 + CHUNK],
                        start=True, stop=True)
                    nc.tensor.matmul(
                        ps[OC:128, :], wt[64:64 + KP, :],
                        i2c[64:64 + KP, c0:c0 + CHUNK],
                        start=True, stop=True)
                    if j % 2 == 0:
                        nc.vector.tensor_copy(ob[:, c0:c0 + CHUNK], ps[:, :])
                    else:
                        nc.scalar.copy(ob[:, c0:c0 + CHUNK], ps[:, :])

                for half in range(2):
                    t = 2 * q + half
                    imgi, blk = divmod(t, NBLK)
                    r0 = BLK * blk
                    nrows = min(BLK, OH - r0)
                    nc.sync.dma_start(
                        out=out[imgi, :, r0:r0 + nrows, :],
                        in_=ob[64 * half:64 * half + OC, :nrows * OW])

    nc.compile()
    return nc


def _prep_inputs(data, weight):
    d = np.asarray(data).reshape(B, H, H).astype(np.float16)
    dpad = np.zeros((B, 256, H), dtype=np.float16)
    dpad[:, :H, :] = d
    dflat = dpad.reshape(B, 256 * H)
    w = np.asarray(weight).reshape(OC, KS * KS).astype(np.float16)

    # wbd[g, ky*KS+kx, oc] = W[oc, ky*KS+kx] for both row groups
    wbd = np.empty((2, KP, OC), dtype=np.float16)
    wbd[0] = w.T
    wbd[1] = w.T

    in_maps = []
    for c in range(NCORES):
        i2cd = np.empty((NTILES, KP, RUN), dtype=np.float16)
        for t in range(NTILES):
            imgi, blk = divmod(t, NBLK)
            g = c * IPC + imgi
            r0 = BLK * blk
            for ky in range(KS):
                base = (r0 + ky) * H
                for kx in range(KS):
                    i2cd[t, ky * KS + kx, :] = \
                        dflat[g, base + kx:base + kx + RUN]
        in_maps.append({"i2cd": i2cd, "wbd": wbd})
    return in_maps


def kernel(data, weight):
    from concourse.bass_utils import run_bass_kernel_spmd

    if "nc" not in _CACHE:
        _CACHE["nc"] = _build()
    nc = _CACHE["nc"]

    in_maps = _prep_inputs(np.asarray(data), np.asarray(weight))
    res = run_bass_kernel_spmd(nc, in_maps, core_ids=list(range(NCORES)))
    outs = [r["out"] for r in res.results]
    full = np.concatenate(outs, axis=0)  # [32, 64, 218, 224] f16
    return np.ascontiguousarray(full[:, :, :, :OH]).astype(np.float32)
